# revision 4
# baseline (speedup 1.0000x reference)
"""Block-diagonal linear (grouped GEMM) on 8 TRN2 NeuronCores.

out[b, g*512+n] = sum_k x[b, g*512+k] * blocks[g, k, n]

Sharding: group-parallel — core g computes block g's GEMM. The host hands
each core xT = bf16(x[:, g*512:(g+1)*512]).T ([512, 8192], feature-major)
and receives bf16 outT ([512, 8192]); transposes and dtype casts happen on
the host so the device streams long contiguous bf16 runs per partition.

Per-core kernel: out.T = W.T @ x.T as PSUM accumulation groups:
psum[n-tile 128, m 512] += W[k-tile, n-tile].T @ xT[k-tile, m-chunk].
All matmul operands are bf16 (1 col/cycle at 2.4 GHz warm — the PE
roofline for this shape, ~54.6 us); accumulation stays fp32 in PSUM and
the result is rounded to bf16 on the PSUM->SBUF copy. End-to-end max rel
err vs the fp32 reference is ~4e-3 (gate: 2e-2).

Head/tail handling: dummy matmuls on a memset scratch tile keep the PE
busy (and the HAM clock-gate warming) while the first chunk + weights
stream in; weights ride the otherwise-idle SWDGE ring so the first x
tiles land sooner; the first/last chunks are small to shorten the
pipeline fill/drain.
"""
import numpy as np
import ml_dtypes

import concourse.bacc as bacc
import concourse.tile as tile
from concourse import mybir
from concourse.bass_utils import run_bass_kernel_spmd

TOKENS = 8192
G = 8
M = 512  # per-block in-features
N = 512  # per-block out-features
P = 128
KT = M // P  # 4 contraction tiles
NT = N // P  # 4 output feature tiles
SUB = 512    # tokens per PSUM group (fp32 PSUM bank = 512 elems)
F32 = mybir.dt.float32
BF16 = mybir.dt.bfloat16
NPBF16 = ml_dtypes.bfloat16

# token-chunk schedule: small head for fast first matmul, small tail for a
# short output drain, 2048 steady state
CHUNKS = [256, 512, 1024, 2048, 2048, 1024, 1024, 256]
assert sum(CHUNKS) == TOKENS
CMAX = max(CHUNKS)
NWARM = 26  # dummy matmuls covering the first-chunk DMA + HAM warmup

_CACHE: dict = {}


def _body(tc, nc, xT, w, outT):
    with (
        tc.tile_pool(name="wp", bufs=1) as wp,
        tc.tile_pool(name="xin", bufs=16) as xin,
        tc.tile_pool(name="outp", bufs=2) as outp,
        tc.tile_pool(name="pso", bufs=7, space="PSUM") as pso,
        tc.tile_pool(name="psw", bufs=1, space="PSUM") as psw,
    ):
        # PE warmup: matmuls on a zeroed scratch tile, queued with no DMA
        # deps so they run while the first chunk streams in. Their PSUM
        # bank is never read.
        scr = wp.tile([P, P], BF16, tag="scr")
        ps_w = psw.tile([P, SUB], F32, tag="psw")
        nc.vector.memset(scr, 0.0)
        for _ in range(NWARM):
            nc.tensor.matmul(ps_w[:, :P], scr, scr, start=True, stop=True)

        # weights [512, 512] bf16 -> [128, kt, 512]: k-tiles 0/1 ride the
        # two HWDGE rings ahead of x, k-tiles 2/3 ride the idle SWDGE ring
        w_r = wp.tile([P, KT, N], BF16, tag="wr")
        w_v = w.rearrange("(j p) n -> j p n", p=P)
        w_eng = [nc.sync, nc.scalar, nc.gpsimd, nc.gpsimd]
        for j in range(KT):
            w_eng[j].dma_start(w_r[:, j, :], w_v[j])

        m0 = 0
        for ci, c in enumerate(CHUNKS):
            # load the 4 k-tiles of this token chunk, striped across the
            # two HWDGE rings (sync=SP and scalar=ACT)
            xs = []
            for j in range(KT):
                x_t = xin.tile([P, CMAX], BF16, tag="x")
                eng = nc.sync if j % 2 == 0 else nc.scalar
                eng.dma_start(x_t[:, :c], xT[j * P:(j + 1) * P, m0:m0 + c])
                xs.append(x_t)

            ots = [outp.tile([P, CMAX], BF16, tag=f"o{nt}", name=f"ot{nt}") for nt in range(NT)]
            for s0 in range(0, c, SUB):
                sw = min(SUB, c - s0)
                for nt in range(NT):
                    ps_o = pso.tile([P, SUB], F32, tag="pso")
                    for j in range(KT):
                        nc.tensor.matmul(
                            ps_o[:, :sw],
                            w_r[:, j, nt * P:(nt + 1) * P],
                            xs[j][:, s0:s0 + sw],
                            start=(j == 0),
                            stop=(j == KT - 1),
                        )
                    nc.vector.tensor_copy(ots[nt][:, s0:s0 + sw], ps_o[:, :sw])
            # flush the chunk: one DMA per n-tile on the SWDGE ring; the last
            # chunks ride the HWDGE rings (input traffic is done by then)
            for nt in range(NT):
                if ci >= len(CHUNKS) - 3:
                    eng = nc.sync if nt % 2 == 0 else nc.scalar
                else:
                    eng = nc.gpsimd
                eng.dma_start(outT[nt * P:(nt + 1) * P, m0:m0 + c], ots[nt][:, :c])
            m0 += c


def _build():
    nc = bacc.Bacc("TRN2", target_bir_lowering=False, debug=False, num_devices=G)
    xT = nc.dram_tensor("xT", [M, TOKENS], BF16, kind="ExternalInput").ap()
    w = nc.dram_tensor("w", [M, N], BF16, kind="ExternalInput").ap()
    outT = nc.dram_tensor("outT", [N, TOKENS], BF16, kind="ExternalOutput").ap()
    with tile.TileContext(nc) as tc:
        _body(tc, nc, xT, w, outT)
    nc.compile()
    return nc


def _run(in_maps, **kwargs):
    if "nc" not in _CACHE:
        _CACHE["nc"] = _build()
    return run_bass_kernel_spmd(_CACHE["nc"], in_maps, list(range(G)), **kwargs)


def _in_maps(x, blocks):
    xb = x.astype(NPBF16)
    wb = blocks.astype(NPBF16)
    return [
        {
            "xT": np.ascontiguousarray(xb[:, g * M:(g + 1) * M].T),
            "w": np.ascontiguousarray(wb[g]),
        }
        for g in range(G)
    ]


def kernel(x, blocks):
    x = np.asarray(x)
    blocks = np.asarray(blocks)
    res = _run(_in_maps(x, blocks))
    return np.concatenate(
        [res.results[g]["outT"].T for g in range(G)], axis=1
    ).astype(np.float32)


# revision 7
# speedup vs baseline: 1.0094x; 1.0094x over previous
"""Block-diagonal linear (grouped GEMM) on 8 TRN2 NeuronCores.

out[b, g*512+n] = sum_k x[b, g*512+k] * blocks[g, k, n]

Sharding: group-parallel — core g computes block g's GEMM. The host hands
each core xT = bf16(x[:, g*512:(g+1)*512]).T ([512, 8192], feature-major)
and receives bf16 outT ([512, 8192]); transposes and dtype casts happen on
the host so the device streams long contiguous bf16 runs per partition.

Per-core kernel: out.T = W.T @ x.T as PSUM accumulation groups:
psum[n-tile 128, m 512] += W[k-tile, n-tile].T @ xT[k-tile, m-chunk].
All matmul operands are bf16 (1 col/cycle at 2.4 GHz warm — the PE
roofline for this shape, ~54.6 us); accumulation stays fp32 in PSUM and
the result is rounded to bf16 on the PSUM->SBUF copy. End-to-end max rel
err vs the fp32 reference is ~4e-3 (gate: 2e-2).

Head/tail handling: dummy matmuls on a memset scratch tile keep the PE
busy (and the HAM clock-gate warming) while the first chunk + weights
stream in; weights ride the otherwise-idle SWDGE ring so the first x
tiles land sooner; the first/last chunks are small to shorten the
pipeline fill/drain.
"""
import numpy as np
import ml_dtypes

import concourse.bacc as bacc
import concourse.tile as tile
from concourse import mybir
from concourse.bass_utils import run_bass_kernel_spmd

TOKENS = 8192
G = 8
M = 512  # per-block in-features
N = 512  # per-block out-features
P = 128
KT = M // P  # 4 contraction tiles
NT = N // P  # 4 output feature tiles
SUB = 512    # tokens per PSUM group (fp32 PSUM bank = 512 elems)
F32 = mybir.dt.float32
BF16 = mybir.dt.bfloat16
NPBF16 = ml_dtypes.bfloat16

# token-chunk schedule: small head for fast first matmul, small tail for a
# short output drain, 2048 steady state
CHUNKS = [512, 512, 1024, 2048, 2048, 1024, 768, 256]
assert sum(CHUNKS) == TOKENS
CMAX = max(CHUNKS)
NWARM = 30  # dummy matmuls covering the first-chunk DMA + HAM warmup

_CACHE: dict = {}


def _body(tc, nc, xT, w, outT):
    with (
        tc.tile_pool(name="wp", bufs=1) as wp,
        tc.tile_pool(name="xin", bufs=16) as xin,
        tc.tile_pool(name="outp", bufs=2) as outp,
        tc.tile_pool(name="pso", bufs=7, space="PSUM") as pso,
        tc.tile_pool(name="psw", bufs=1, space="PSUM") as psw,
    ):
        # PE warmup: matmuls on a zeroed scratch tile, queued with no DMA
        # deps so they run while the first chunk streams in. Their PSUM
        # bank is never read.
        scr = wp.tile([P, P], BF16, tag="scr")
        ps_w = psw.tile([P, SUB], F32, tag="psw")
        nc.vector.memset(scr, 0.0)
        for _ in range(NWARM):
            nc.tensor.matmul(ps_w[:, :P], scr, scr, start=True, stop=True)

        # weights [512, 512] bf16 -> [128, kt, 512]: k-tiles 0/1 ride the
        # two HWDGE rings ahead of x, k-tiles 2/3 ride the idle SWDGE ring
        w_r = wp.tile([P, KT, N], BF16, tag="wr")
        w_v = w.rearrange("(j p) n -> j p n", p=P)
        w_eng = [nc.sync, nc.scalar, nc.gpsimd, nc.gpsimd]
        for j in range(KT):
            w_eng[j].dma_start(w_r[:, j, :], w_v[j])

        m0 = 0
        for ci, c in enumerate(CHUNKS):
            # load the 4 k-tiles of this token chunk, striped across the
            # two HWDGE rings (sync=SP and scalar=ACT)
            xs = []
            for j in range(KT):
                x_t = xin.tile([P, CMAX], BF16, tag="x")
                eng = nc.sync if j % 2 == 0 else nc.scalar
                eng.dma_start(x_t[:, :c], xT[j * P:(j + 1) * P, m0:m0 + c])
                xs.append(x_t)

            ots = [outp.tile([P, CMAX], BF16, tag=f"o{nt}", name=f"ot{nt}") for nt in range(NT)]
            if ci == 0:
                # k-outer accumulation: the first 2*NT matmuls need only
                # k-tiles 0/1 (the HWDGE rings), which land ~1.5us before
                # k-tiles 2/3 arrive behind the weights on the SWDGE ring
                pss = [pso.tile([P, SUB], F32, tag="pso", name=f"ps{nt}") for nt in range(NT)]
                for j in range(KT):
                    for nt in range(NT):
                        nc.tensor.matmul(
                            pss[nt][:, :c],
                            w_r[:, j, nt * P:(nt + 1) * P],
                            xs[j][:, :c],
                            start=(j == 0),
                            stop=(j == KT - 1),
                        )
                for nt in range(NT):
                    nc.vector.tensor_copy(ots[nt][:, :c], pss[nt][:, :c])
            else:
                for s0 in range(0, c, SUB):
                    sw = min(SUB, c - s0)
                    for nt in range(NT):
                        ps_o = pso.tile([P, SUB], F32, tag="pso")
                        for j in range(KT):
                            nc.tensor.matmul(
                                ps_o[:, :sw],
                                w_r[:, j, nt * P:(nt + 1) * P],
                                xs[j][:, s0:s0 + sw],
                                start=(j == 0),
                                stop=(j == KT - 1),
                            )
                        nc.vector.tensor_copy(ots[nt][:, s0:s0 + sw], ps_o[:, :sw])
                        if ci >= len(CHUNKS) - 2:
                            # drain the tail early: flush each PSUM-group
                            # slice as soon as it is cast, on the HWDGE rings
                            eng = nc.sync if nt % 2 == 0 else nc.scalar
                            eng.dma_start(
                                outT[nt * P:(nt + 1) * P, m0 + s0:m0 + s0 + sw],
                                ots[nt][:, s0:s0 + sw],
                            )
            # flush the chunk: one DMA per n-tile on the SWDGE ring; the
            # third-from-last rides the HWDGE rings (input traffic is done
            # by then); the last two were flushed per-SUB above
            if ci < len(CHUNKS) - 2:
                for nt in range(NT):
                    if ci == len(CHUNKS) - 3:
                        eng = nc.sync if nt % 2 == 0 else nc.scalar
                    else:
                        eng = nc.gpsimd
                    eng.dma_start(outT[nt * P:(nt + 1) * P, m0:m0 + c], ots[nt][:, :c])
            m0 += c


def _build():
    nc = bacc.Bacc("TRN2", target_bir_lowering=False, debug=False, num_devices=G)
    xT = nc.dram_tensor("xT", [M, TOKENS], BF16, kind="ExternalInput").ap()
    w = nc.dram_tensor("w", [M, N], BF16, kind="ExternalInput").ap()
    outT = nc.dram_tensor("outT", [N, TOKENS], BF16, kind="ExternalOutput").ap()
    with tile.TileContext(nc) as tc:
        _body(tc, nc, xT, w, outT)
    nc.compile()
    return nc


def _run(in_maps, **kwargs):
    if "nc" not in _CACHE:
        _CACHE["nc"] = _build()
    return run_bass_kernel_spmd(_CACHE["nc"], in_maps, list(range(G)), **kwargs)


def _in_maps(x, blocks):
    xb = x.astype(NPBF16)
    wb = blocks.astype(NPBF16)
    return [
        {
            "xT": np.ascontiguousarray(xb[:, g * M:(g + 1) * M].T),
            "w": np.ascontiguousarray(wb[g]),
        }
        for g in range(G)
    ]


def kernel(x, blocks):
    x = np.asarray(x)
    blocks = np.asarray(blocks)
    res = _run(_in_maps(x, blocks))
    return np.concatenate(
        [res.results[g]["outT"].T for g in range(G)], axis=1
    ).astype(np.float32)


# revision 8
# speedup vs baseline: 1.0395x; 1.0298x over previous
"""Block-diagonal linear (grouped GEMM) on 8 TRN2 NeuronCores.

out[b, g*512+n] = sum_k x[b, g*512+k] * blocks[g, k, n]

Sharding: group-parallel — core g computes block g's GEMM. The host hands
each core xT = bf16(x[:, g*512:(g+1)*512]).T ([512, 8192], feature-major)
and receives bf16 outT ([512, 8192]); transposes and dtype casts happen on
the host so the device streams long contiguous bf16 runs per partition.

Per-core kernel: out.T = W.T @ x.T as PSUM accumulation groups:
psum[n-tile 128, m 512] += W[k-tile, n-tile].T @ xT[k-tile, m-chunk].
All matmul operands are bf16 (1 col/cycle at 2.4 GHz warm — the PE
roofline for this shape, ~54.6 us); accumulation stays fp32 in PSUM and
the result is rounded to bf16 on the PSUM->SBUF copy. End-to-end max rel
err vs the fp32 reference is ~4e-3 (gate: 2e-2).

Schedule notes (from NTFF traces): the kernel is PE-roofline-bound, so
the fight is head/tail latency. Dummy matmuls on a memset scratch tile
keep the PE busy (and the HAM clock-gate warming) while the first chunk
streams in; each chunk's 4 k-tiles ride as ONE DMA per HWDGE ring
(k-tiles 0/1 on SP, 2/3 on ACT via a strided AP) because every dma_start
costs ~600ns of descriptor generation plus a multi-us 16-engine
completion-semaphore tail; the SWDGE ring is kept idle during the head
(its packets contend for the same 16 SDMA engines) and carries the bulk
output later; the first/last chunks are small to shorten fill/drain.
"""
import numpy as np
import ml_dtypes

import concourse.bacc as bacc
import concourse.tile as tile
from concourse import mybir
from concourse.bass_utils import run_bass_kernel_spmd

TOKENS = 8192
G = 8
M = 512  # per-block in-features
N = 512  # per-block out-features
P = 128
KT = M // P  # 4 contraction tiles
NT = N // P  # 4 output feature tiles
SUB = 512    # tokens per PSUM group (fp32 PSUM bank = 512 elems)
F32 = mybir.dt.float32
BF16 = mybir.dt.bfloat16
NPBF16 = ml_dtypes.bfloat16

# token-chunk schedule: small head for fast first matmul, small tail for a
# short output drain, 2048 steady state
CHUNKS = [256, 256, 512, 1024, 2048, 2048, 1024, 768, 256]
assert sum(CHUNKS) == TOKENS
CMAX = max(CHUNKS)
NWARM = 20   # dummy matmuls (N=256) covering first-chunk DMA + HAM warmup

_CACHE: dict = {}


def _body(tc, nc, xT, w, outT):
    with (
        tc.tile_pool(name="wp", bufs=1) as wp,
        tc.tile_pool(name="xin", bufs=8) as xin,
        tc.tile_pool(name="outp", bufs=2) as outp,
        tc.tile_pool(name="pso", bufs=7, space="PSUM") as pso,
        tc.tile_pool(name="psw", bufs=1, space="PSUM") as psw,
    ):
        # PE warmup: matmuls on a zeroed scratch tile, queued with no DMA
        # deps so they run while the first chunk streams in. Their PSUM
        # bank is never read.
        scr = wp.tile([P, 2 * P], BF16, tag="scr")
        ps_w = psw.tile([P, SUB], F32, tag="psw")
        nc.vector.memset(scr, 0.0)
        for _ in range(NWARM):
            nc.tensor.matmul(ps_w[:, :2 * P], scr[:, :P], scr, start=True, stop=True)

        # weights [512, 512] bf16 -> [128, kt, 512], one half-DMA per
        # HWDGE ring, queued ahead of the x stream
        w_r = wp.tile([P, KT, N], BF16, tag="wr")
        w_v = w.rearrange("(j p) n -> p j n", p=P)
        nc.sync.dma_start(w_r[:, 0:2, :], w_v[:, 0:2, :])
        nc.scalar.dma_start(w_r[:, 2:4, :], w_v[:, 2:4, :])

        x_v = xT.rearrange("(j p) t -> p j t", p=P)
        m0 = 0
        for ci, c in enumerate(CHUNKS):
            # one DMA per ring per chunk: k-tiles 0/1 on SP, 2/3 on ACT
            x01 = xin.tile([P, 2, CMAX], BF16, tag="x01")
            x23 = xin.tile([P, 2, CMAX], BF16, tag="x23")
            nc.sync.dma_start(x01[:, :, :c], x_v[:, 0:2, m0:m0 + c])
            nc.scalar.dma_start(x23[:, :, :c], x_v[:, 2:4, m0:m0 + c])
            xs = [x01[:, 0, :], x01[:, 1, :], x23[:, 0, :], x23[:, 1, :]]

            ots = [outp.tile([P, CMAX], BF16, tag=f"o{nt}", name=f"ot{nt}") for nt in range(NT)]
            for s0 in range(0, c, SUB):
                sw = min(SUB, c - s0)
                for nt in range(NT):
                    ps_o = pso.tile([P, SUB], F32, tag="pso")
                    for j in range(KT):
                        nc.tensor.matmul(
                            ps_o[:, :sw],
                            w_r[:, j, nt * P:(nt + 1) * P],
                            xs[j][:, s0:s0 + sw],
                            start=(j == 0),
                            stop=(j == KT - 1),
                        )
                    nc.vector.tensor_copy(ots[nt][:, s0:s0 + sw], ps_o[:, :sw])
                    if ci >= len(CHUNKS) - 2:
                        # drain the tail early: flush each PSUM-group slice
                        # as soon as it is cast, on the HWDGE rings
                        eng = nc.sync if nt % 2 == 0 else nc.scalar
                        eng.dma_start(
                            outT[nt * P:(nt + 1) * P, m0 + s0:m0 + s0 + sw],
                            ots[nt][:, s0:s0 + sw],
                        )
            # flush the chunk: one DMA per n-tile on the SWDGE ring (idle
            # during the head; the HWDGE rings carry all input); the last
            # two chunks were flushed per-SUB above
            if ci < len(CHUNKS) - 2:
                for nt in range(NT):
                    nc.gpsimd.dma_start(
                        outT[nt * P:(nt + 1) * P, m0:m0 + c], ots[nt][:, :c]
                    )
            m0 += c


def _build():
    nc = bacc.Bacc("TRN2", target_bir_lowering=False, debug=False, num_devices=G)
    xT = nc.dram_tensor("xT", [M, TOKENS], BF16, kind="ExternalInput").ap()
    w = nc.dram_tensor("w", [M, N], BF16, kind="ExternalInput").ap()
    outT = nc.dram_tensor("outT", [N, TOKENS], BF16, kind="ExternalOutput").ap()
    with tile.TileContext(nc) as tc:
        _body(tc, nc, xT, w, outT)
    nc.compile()
    return nc


def _run(in_maps, **kwargs):
    if "nc" not in _CACHE:
        _CACHE["nc"] = _build()
    return run_bass_kernel_spmd(_CACHE["nc"], in_maps, list(range(G)), **kwargs)


def _in_maps(x, blocks):
    xb = x.astype(NPBF16)
    wb = blocks.astype(NPBF16)
    return [
        {
            "xT": np.ascontiguousarray(xb[:, g * M:(g + 1) * M].T),
            "w": np.ascontiguousarray(wb[g]),
        }
        for g in range(G)
    ]


def kernel(x, blocks):
    x = np.asarray(x)
    blocks = np.asarray(blocks)
    res = _run(_in_maps(x, blocks))
    return np.concatenate(
        [res.results[g]["outT"].T for g in range(G)], axis=1
    ).astype(np.float32)
